# revision 11
# baseline (speedup 1.0000x reference)
"""Multi-head attention (B=2, S=2048, D=1024, H=16, RoPE) on 8 Trainium2 cores.

Sharding: tensor-parallel over heads. Core c owns heads (2c, 2c+1):
 - W_qkv column-sliced to that head pair (q|k|v blocks of 128 cols each),
 - W_out row-sliced to the pair's 128 input dims,
 - every core reads all tokens (x shipped pre-transposed as x^T, bf16),
 - each core emits a partial [4096, 1024] bf16 output; host sums the 8
   partials in f32 and adds b_out (Megatron-style allreduce on host).

Device program (per core, identical SPMD; all matmul operands bf16 so the
PE streams 1 row/cycle and LDWEIGHTS uses fast-weight-load):
  QKV runs in 512-token groups with the weight slice stationary across the
  whole group (9 accumulating matmuls incl. a ones-row bias matmul). RoPE =
  ptab-matmul rotate + two DVE multiply-adds against bf16 cos/sin tables.
  V^T is produced by the DMA XBAR transpose into a staging tile, then two
  strided DVE copies place it in the V2 layout
  [VA(64) | 1 | pad | 1 | 0(63) | VB(64)] (group width 194) so the two
  attn@V matmuls per key block also produce the softmax denominators:
  l_A lands on PSUM row 64 (cols 0:512), l_B on row 0 (cols 512:1024).
  Attention per (batch, 512-query chunk) pipelines 128-key blocks:
  score matmuls use 64-partition operands (no zero-padded K tiles), one
  batched exp on ACT with the 1/8 scale folded in, attn@V accumulated in
  PSUM. The merge avoids any DRAM bounce: one tiny SBUF->SBUF DMA hops
  l_A to partition 0, reciprocal_approx_fast + one partition_broadcast
  give 1/l, and two bf16 multiplies build the outproj stationary a2.
  Batch-1 QKV and the output projections are interleaved into the
  attention key-block slots through a task queue so the PE never idles.
"""

import sys

if "/opt/trn_rl_repo" not in sys.path:
    sys.path.insert(0, "/opt/trn_rl_repo")

import numpy as np
import ml_dtypes

import concourse.bacc as bacc
import concourse.mybir as mybir
from concourse.tile import TileContext
from concourse.bass_utils import run_bass_kernel_spmd

F32 = mybir.dt.float32
BF16 = mybir.dt.bfloat16
BF = ml_dtypes.bfloat16
ADD = mybir.AluOpType.add
MUL = mybir.AluOpType.mult
EXP = mybir.ActivationFunctionType.Exp

B, S, D, H, DH = 2, 2048, 1024, 16, 64
S2 = B * S              # 4096 tokens total
G = 512                 # token group for the projection phase
GPB = S // G            # 4 groups per batch
NSC = 4                 # 512-query chunks per batch
NTB = S // 128          # 16 key blocks per batch
VG = 194                # V2 group: VA(64)|1|pad|1|zeros(63)|VB(64)


def _build_program():
    nc = bacc.Bacc("TRN2", target_bir_lowering=False, debug=False, num_devices=8)

    xT = nc.dram_tensor("xT", [D, S2], BF16, kind="ExternalInput")
    W = nc.dram_tensor("W", [D, 384], BF16, kind="ExternalInput")
    bq = nc.dram_tensor("bq", [1, 384], BF16, kind="ExternalInput")
    Wo = nc.dram_tensor("Wo", [128, 1024], BF16, kind="ExternalInput")
    ctab_d = nc.dram_tensor("ctab", [128, S], BF16, kind="ExternalInput")
    stab_d = nc.dram_tensor("stab", [128, S], BF16, kind="ExternalInput")
    ptab_d = nc.dram_tensor("ptab", [128, 128], BF16, kind="ExternalInput")
    out_d = nc.dram_tensor("out", [S2, D], BF16, kind="ExternalOutput")

    xT_re = xT.rearrange("(kb p) n -> p kb n", p=128)   # [128, 8, 4096]
    W_re = W.rearrange("(kb p) m -> p kb m", p=128)     # [128, 8, 384]

    with TileContext(nc) as tc:
        with tc.tile_pool(name="consts", bufs=1) as cp, \
             tc.tile_pool(name="xg", bufs=6) as xgp, \
             tc.tile_pool(name="pre", bufs=2) as prep, \
             tc.tile_pool(name="tmp", bufs=4) as tmpp, \
             tc.tile_pool(name="vt", bufs=2) as vtp, \
             tc.tile_pool(name="vv", bufs=2) as vvp, \
             tc.tile_pool(name="pa", bufs=3) as ptp, \
             tc.tile_pool(name="mrg", bufs=2) as mrgp, \
             tc.tile_pool(name="la", bufs=2) as lap, \
             tc.tile_pool(name="rc", bufs=2) as rcp, \
             tc.tile_pool(name="rlb", bufs=2) as rlbp, \
             tc.tile_pool(name="a2", bufs=2) as a2p, \
             tc.tile_pool(name="osb", bufs=2) as osbp, \
             tc.tile_pool(name="ps5", bufs=2, space="PSUM") as qkps, \
             tc.tile_pool(name="pssc", bufs=2, space="PSUM") as pssc, \
             tc.tile_pool(name="gam", bufs=1, space="PSUM") as gamp:

            W_r = cp.tile([128, 8 * 384], BF16, tag="W_r")
            Wo_r = cp.tile([128, 1024], BF16, tag="Wo_r")
            ctab = cp.tile([128, S], BF16, tag="ctab")
            stab = cp.tile([128, S], BF16, tag="stab")
            ptab = cp.tile([128, 128], BF16, tag="ptab")
            bq_r = cp.tile([128, 384], BF16, tag="bq_r")
            ones_r = cp.tile([128, G], BF16, tag="ones_r")
            qTb = [cp.tile([128, S], BF16, tag=f"qT{b}", name=f"qT{b}")
                   for b in range(B)]
            kTb = [cp.tile([128, S], BF16, tag=f"kT{b}", name=f"kT{b}")
                   for b in range(B)]
            V2b = [cp.tile([128, NTB * VG], BF16, tag=f"V2{b}", name=f"V2{b}")
                   for b in range(B)]

            def emit_setup_dmas():
                nc.scalar.dma_start(out=ctab[:], in_=ctab_d[:])
                nc.scalar.dma_start(out=stab[:], in_=stab_d[:])
                nc.scalar.dma_start(out=ptab[:], in_=ptab_d[:])
                nc.scalar.dma_start(out=Wo_r[:], in_=Wo[:])
                nc.scalar.dma_start(out=bq_r[0:1, :], in_=bq[:])

            nc.gpsimd.memset(bq_r[:], 0.0)
            nc.gpsimd.memset(ones_r[:], 0.0)
            nc.gpsimd.memset(ones_r[0:1, :], 1.0)
            for b in range(B):
                v2v = V2b[b][:].rearrange("p (g c) -> p g c", g=NTB)
                nc.gpsimd.memset(v2v[:, :, 64:65], 1.0)
                nc.gpsimd.memset(v2v[:, :, 65:66], 0.0)
                nc.gpsimd.memset(v2v[:, :, 66:67], 1.0)
                nc.gpsimd.memset(v2v[:, :, 67:130], 0.0)

            # ---------------- emitters ----------------------------------
            def emit_xg_load(b, g, eng=None):
                tok0 = b * S + g * G
                xg = xgp.tile([128, 8 * G], BF16, tag="xg", name=f"xg{b}_{g}")
                (eng or nc.sync).dma_start(
                    out=xg[:].rearrange("p (kb n) -> p kb n", kb=8),
                    in_=xT_re[:, :, tok0:tok0 + G])
                return xg

            def emit_qkv_mt(b, g, xg, mt, defer_v=None):
                scol = g * G
                ps = qkps.tile([128, G], F32, tag="ps5", name=f"qkv{b}{g}{mt}")
                for kb in range(8):
                    c0 = kb * 384 + mt * 128
                    nc.tensor.matmul(
                        ps[:], W_r[:, c0:c0 + 128], xg[:, kb * G:(kb + 1) * G],
                        start=(kb == 0), stop=False)
                nc.tensor.matmul(
                    ps[:], bq_r[:, mt * 128:(mt + 1) * 128], ones_r[:],
                    start=False, stop=True)
                if mt < 2:
                    pre = prep.tile([128, G], BF16, tag="pre",
                                    name=f"pre{b}{g}{mt}")
                    nc.vector.tensor_copy(pre[:], ps[:])
                    rot = qkps.tile([128, G], F32, tag="ps5",
                                    name=f"rot{b}{g}{mt}")
                    nc.tensor.matmul(rot[:], ptab[:], pre[:],
                                     start=True, stop=True)
                    t1 = tmpp.tile([128, G], BF16, tag="tmp",
                                   name=f"t1{b}{g}{mt}")
                    nc.vector.tensor_tensor(
                        out=t1[:], in0=rot[:], in1=stab[:, scol:scol + G],
                        op=MUL)
                    t2 = tmpp.tile([128, G], BF16, tag="tmp",
                                   name=f"t2{b}{g}{mt}")
                    nc.vector.tensor_tensor(
                        out=t2[:], in0=pre[:], in1=ctab[:, scol:scol + G],
                        op=MUL)
                    dst = (qTb if mt == 0 else kTb)[b]
                    nc.vector.tensor_tensor(
                        out=dst[:, scol:scol + G], in0=t1[:], in1=t2[:],
                        op=ADD)
                else:
                    vt = vtp.tile([128, G], BF16, tag="vt", name=f"vt{b}{g}")
                    nc.vector.tensor_copy(vt[:], ps[:])

                    def emit_vplace(b=b, g=g, vt=vt, eng=None):
                        vv = vvp.tile([128, G], BF16, tag="vv",
                                      name=f"vv{b}{g}")
                        (eng or nc.sync).dma_start(
                            out=vv[:].rearrange("p (g c) -> p g c", g=4),
                            in_=vt[:], transpose=True)
                        vvv = vv[:].rearrange("p (g c) -> p g c", g=4)
                        v2v = V2b[b][:].rearrange("p (g c) -> p g c", g=NTB)
                        g4 = g * 4
                        nc.vector.tensor_copy(
                            v2v[:, g4:g4 + 4, 0:64], vvv[:, :, 0:64])
                        nc.vector.tensor_copy(
                            v2v[:, g4:g4 + 4, 130:194], vvv[:, :, 64:128])
                    if defer_v is not None:
                        defer_v.append(emit_vplace)
                    else:
                        emit_vplace()

            heavy = []          # batch-1 qkv emitters (~2us PE each)
            light = []          # outproj units + stores (~0.5us each)
            HEAVY_SLOTS = frozenset((3, 7, 11, 14))
            LIGHT_SLOTS = frozenset((2, 5, 9, 13, 15))

            def emit_sc(b, qcol, qw, uid, stage_at=(), box=None):
                if stage_at:
                    gen = _emit_sc_gen(b, qcol, qw, uid, stage_at, box)
                    return gen
                g = _emit_sc_gen(b, qcol, qw, uid, (), box)
                for _ in g:
                    pass
                return box[0] if box else None

            def _emit_sc_gen(b, qcol, qw, uid, stage_at, box):
                qT, kT, V2 = qTb[b], kTb[b], V2b[b]
                gam = gamp.tile([128, 1024], F32, tag="gam",
                                name=f"gam{uid}")

                def av(tb, pa):
                    gcol = tb * VG
                    st, sp = (tb == 0), (tb == NTB - 1)
                    nc.tensor.matmul(
                        gam[0:65, 0:qw], V2[:, gcol:gcol + 65],
                        pa[:, 0:qw], start=st, stop=sp)
                    nc.tensor.matmul(
                        gam[:, 512:512 + qw], V2[:, gcol + 66:gcol + 194],
                        pa[:, 512:512 + qw], start=st, stop=sp)

                prev = None
                for tb in range(NTB):
                    if tb in stage_at:
                        yield tb
                    tcol = tb * 128
                    sco = pssc.tile([128, 1024], F32, tag="sco",
                                    name=f"sco{uid}{tb}")
                    nc.tensor.matmul(
                        sco[:, 0:qw], kT[0:64, tcol:tcol + 128],
                        qT[0:64, qcol:qcol + qw], start=True, stop=True)
                    nc.tensor.matmul(
                        sco[:, 512:512 + qw], kT[64:128, tcol:tcol + 128],
                        qT[64:128, qcol:qcol + qw], start=True, stop=True)
                    pa = ptp.tile([128, 1024], BF16, tag="pa",
                                  name=f"pa{uid}{tb}")
                    scv = sco[:].rearrange("p (h q) -> p h q", h=2)
                    pav = pa[:].rearrange("p (h q) -> p h q", h=2)
                    nc.scalar.activation(
                        pav[:, :, 0:qw], scv[:, :, 0:qw], EXP, scale=0.125)
                    if prev is not None:
                        av(*prev)
                    if tb in HEAVY_SLOTS and heavy:
                        heavy.pop(0)()
                    elif (tb in LIGHT_SLOTS or not heavy) and light \
                            and tb >= (2 if heavy else 4):
                        light.pop(0)()
                    prev = (tb, pa)
                av(*prev)

                # merge: attnA rows 0:64 (l_A at row 64, cols 0:qw),
                # attnB rows 64:128 (l_B at row 0, cols 512:512+qw)
                s_t = mrgp.tile([128, 1024], F32, tag="s_t",
                                name=f"s_t{uid}")
                nc.vector.tensor_copy(s_t[:, 0:qw], gam[:, 0:qw])
                nc.vector.tensor_copy(
                    s_t[:, 512:512 + qw], gam[:, 512:512 + qw])
                la0 = lap.tile([1, 512], F32, tag="la", name=f"la{uid}")
                nc.sync.dma_start(out=la0[0:1, 0:qw], in_=s_t[64:65, 0:qw])
                rc = rcp.tile([1, 1024], F32, tag="rc", name=f"rc{uid}")
                nc.vector.reciprocal_approx_fast(
                    out=rc[0:1, 0:qw], in_=la0[0:1, 0:qw])
                nc.vector.reciprocal_approx_fast(
                    out=rc[0:1, 512:512 + qw], in_=s_t[0:1, 512:512 + qw])
                rlb = rlbp.tile([128, 1024], F32, tag="rlb",
                                name=f"rlb{uid}")
                rcv = rc[:].rearrange("p (h q) -> p h q", h=2)
                rlbv = rlb[:].rearrange("p (h q) -> p h q", h=2)
                nc.gpsimd.partition_broadcast(
                    out_ap=rlbv[:, :, 0:qw], in_ap=rcv[:, :, 0:qw])
                a2 = a2p.tile([128, 512], BF16, tag="a2", name=f"a2{uid}")
                nc.vector.tensor_tensor(
                    out=a2[0:64, 0:qw], in0=s_t[0:64, 0:qw],
                    in1=rlb[0:64, 0:qw], op=MUL)
                nc.vector.tensor_tensor(
                    out=a2[64:128, 0:qw], in0=s_t[64:128, 512:512 + qw],
                    in1=rlb[64:128, 512:512 + qw], op=MUL)
                if box is not None:
                    box.append(a2)
                yield NTB

            def make_outproj(b, qcol, qw, a2, uid):
                nnb = qw // 128
                osb = osbp.tile([128, 4 * 1024], BF16, tag="osb",
                                name=f"osb{uid}")
                osbv = osb[:].rearrange("p (g c) -> p g c", g=4)

                def emit_nb(nb):
                    for jc in range(2):
                        om = qkps.tile([128, 512], F32, tag="ps5",
                                       name=f"om{uid}{nb}{jc}")
                        nc.tensor.matmul(
                            om[:], a2[:, nb * 128:(nb + 1) * 128],
                            Wo_r[:, jc * 512:(jc + 1) * 512],
                            start=True, stop=True)
                        nc.vector.tensor_copy(
                            osbv[:, nb, jc * 512:(jc + 1) * 512], om[:])

                def emit_store():
                    r0 = b * S + qcol
                    nc.sync.dma_start(
                        out=out_d[r0:r0 + qw, :].rearrange(
                            "(g p) d -> p g d", p=128),
                        in_=osbv[:, 0:nnb, :])

                return [lambda nb=nb: emit_nb(nb) for nb in range(nnb)] \
                    + [emit_store]

            # ---------------- schedule ----------------------------------
            # batch 0 projections up front; xg0 rides the scalar HWDGE
            # queue so it transfers in parallel with W on the sync queue
            defer_v = []
            nc.sync.dma_start(
                out=W_r[:].rearrange("p (kb m) -> p kb m", kb=8),
                in_=W_re[:])
            xg_next = emit_xg_load(0, 0, eng=nc.scalar)
            emit_setup_dmas()

            # batch-0 projections fused with the first attention chunk:
            # after group g's qkv, emit sc0's key blocks 4g..4g+4 so the
            # scalar engine starts softmax work ~15us into the run
            box0 = []
            gen0 = emit_sc(0, 0, 512, uid="c0", stage_at=(4, 8, 12),
                           box=box0)
            xg1 = []
            for g in range(GPB):
                xg = xg_next
                if g + 1 < GPB:
                    xg_next = emit_xg_load(0, g + 1)
                for mt in range(3):
                    emit_qkv_mt(0, g, xg, mt, defer_v=defer_v)
                defer_v.pop(0)(eng=nc.scalar)   # V tiles via scalar queue
                if g == GPB - 1:
                    # batch-1 prefetch must hit the sync queue before
                    # sc0's merge DMA (which waits on compute)
                    xg1 = [emit_xg_load(1, gg) for gg in range(GPB)]
                next(gen0)
            for _ in gen0:
                pass
            light.extend(make_outproj(0, 0, 512, box0[0], uid="c0"))

            for g in range(GPB):
                for mt in range(3):
                    heavy.append(
                        lambda g=g, mt=mt: emit_qkv_mt(1, g, xg1[g], mt))

            # remaining attention chunks; batch-1 projections and the
            # output projections ride the key-block task slots; the
            # final query chunks shrink so the closing tail is short
            chunks = [(0, sc * 512, 512) for sc in range(1, NSC)] \
                + [(1, sc * 512, 512) for sc in range(NSC - 1)] \
                + [(1, 1536, 256), (1, 1792, 256)]
            for i, (b, qcol, qw) in enumerate(chunks):
                box = []
                emit_sc(b, qcol, qw, uid=f"c{i}", box=box)
                light.extend(make_outproj(b, qcol, qw, box[0], uid=f"c{i}"))

            for fn in heavy + light:
                fn()
            heavy.clear()
            light.clear()

    nc.compile()
    return nc


_PROG = None


def _get_program():
    global _PROG
    if _PROG is None:
        _PROG = _build_program()
    return _PROG


def _rope_tables():
    inv_freq = (1.0 / (10000.0 ** (np.arange(0, DH, 2, dtype=np.float32) / DH)))
    invf2 = inv_freq[np.arange(128) % 32]
    ang = np.arange(S, dtype=np.float32)[None, :] * invf2[:, None]
    return np.cos(ang).astype(BF), np.sin(ang).astype(BF)


def _ptab():
    p = np.zeros((128, 128), dtype=np.float32)
    j = np.arange(128)
    p[j ^ 32, j] = np.where((j % 64) < 32, -1.0, 1.0)
    return p.astype(BF)


def make_in_maps(x, W_qkv, b_qkv, W_out, b_out):
    x = np.asarray(x, dtype=np.float32)
    W_qkv = np.asarray(W_qkv, dtype=np.float32)
    b_qkv = np.asarray(b_qkv, dtype=np.float32)
    W_out = np.asarray(W_out, dtype=np.float32)

    xT = np.ascontiguousarray(x.reshape(S2, D).T).astype(BF)
    ct, st = _rope_tables()
    pt = _ptab()

    in_maps = []
    for c in range(8):
        hA, hB = 2 * c, 2 * c + 1
        cols = np.r_[hA * DH:(hA + 1) * DH, hB * DH:(hB + 1) * DH]
        Wc = np.ascontiguousarray(
            np.concatenate([W_qkv[:, off + cols] for off in (0, D, 2 * D)],
                           axis=1)).astype(BF)
        bqc = np.concatenate(
            [b_qkv[off + cols] for off in (0, D, 2 * D)])[None, :].astype(BF)
        Woc = np.ascontiguousarray(W_out[c * 128:(c + 1) * 128, :]).astype(BF)
        in_maps.append({"xT": xT, "W": Wc, "bq": bqc, "Wo": Woc,
                       "ctab": ct, "stab": st, "ptab": pt})
    return in_maps


def assemble_output(results, b_out):
    acc = np.asarray(results[0]["out"]).astype(np.float32)
    for c in range(1, 8):
        acc += np.asarray(results[c]["out"]).astype(np.float32)
    out = acc + np.asarray(b_out, dtype=np.float32)
    return out.reshape(B, S, D).astype(np.float32)


def kernel(x, W_qkv, b_qkv, W_out, b_out):
    nc = _get_program()
    in_maps = make_in_maps(x, W_qkv, b_qkv, W_out, b_out)
    res = run_bass_kernel_spmd(nc, in_maps, core_ids=list(range(8)))
    return assemble_output(res.results, b_out)


if __name__ == "__main__":
    rng = np.random.default_rng(0)
    ins = {
        "x": rng.standard_normal((B, S, D), dtype=np.float32),
        "W_qkv": rng.standard_normal((D, 3 * D), dtype=np.float32) / 32.0,
        "b_qkv": np.zeros(3 * D, np.float32),
        "W_out": rng.standard_normal((D, D), dtype=np.float32) / 32.0,
        "b_out": np.zeros(D, np.float32),
    }
    o = kernel(**ins)
    print("kernel ran:", o.shape, o.dtype)


# revision 12
# speedup vs baseline: 1.2467x; 1.2467x over previous
"""Multi-head attention (B=2, S=2048, D=1024, H=16, RoPE) on 8 Trainium2 cores.

Sharding: tensor-parallel over heads. Core c owns heads (2c, 2c+1):
 - W_qkv column-sliced to that head pair (q|k|v blocks of 128 cols each),
 - W_out row-sliced to the pair's 128 input dims,
 - every core reads all tokens (x shipped pre-transposed as x^T, bf16),
 - each core emits a partial [4096, 1024] bf16 output; host sums the 8
   partials in f32 and adds b_out (Megatron-style allreduce on host).

Device program (per core, identical SPMD; all matmul operands bf16 so the
PE streams 1 row/cycle and LDWEIGHTS uses fast-weight-load):
  QKV runs in 512-token groups with the weight slice stationary across the
  whole group (9 accumulating matmuls incl. a ones-row bias matmul). RoPE =
  ptab-matmul rotate + two DVE multiply-adds against bf16 cos/sin tables.
  V^T is produced by the DMA XBAR transpose into a staging tile, then two
  strided DVE copies place it in the V2 layout
  [VA(64) | 1 | pad | 1 | 0(63) | VB(64)] (group width 194) so the two
  attn@V matmuls per key block also produce the softmax denominators:
  l_A lands on PSUM row 64 (cols 0:512), l_B on row 0 (cols 512:1024).
  Attention per (batch, 512-query chunk) pipelines 128-key blocks:
  score matmuls use 64-partition operands (no zero-padded K tiles), one
  batched exp on ACT with the 1/8 scale folded in, attn@V accumulated in
  PSUM. The merge avoids any DRAM bounce: one tiny SBUF->SBUF DMA hops
  l_A to partition 0, reciprocal_approx_fast + one partition_broadcast
  give 1/l, and two bf16 multiplies build the outproj stationary a2.
  Batch-1 QKV and the output projections are interleaved into the
  attention key-block slots through a task queue so the PE never idles.
"""

import sys

if "/opt/trn_rl_repo" not in sys.path:
    sys.path.insert(0, "/opt/trn_rl_repo")

import numpy as np
import ml_dtypes

import concourse.bacc as bacc
import concourse.mybir as mybir
from concourse.tile import TileContext
from concourse.bass_utils import run_bass_kernel_spmd

F32 = mybir.dt.float32
BF16 = mybir.dt.bfloat16
BF = ml_dtypes.bfloat16
ADD = mybir.AluOpType.add
MUL = mybir.AluOpType.mult
EXP = mybir.ActivationFunctionType.Exp

B, S, D, H, DH = 2, 2048, 1024, 16, 64
S2 = B * S              # 4096 tokens total
G = 512                 # token group for the projection phase
GPB = S // G            # 4 groups per batch
NSC = 4                 # 512-query chunks per batch
NTB = S // 128          # 16 key blocks per batch
VG = 194                # V2 group: VA(64)|1|pad|1|zeros(63)|VB(64)


def _build_program():
    nc = bacc.Bacc("TRN2", target_bir_lowering=False, debug=False, num_devices=8)

    xT = nc.dram_tensor("xT", [D, S2], BF16, kind="ExternalInput")
    W = nc.dram_tensor("W", [D, 384], BF16, kind="ExternalInput")
    bq = nc.dram_tensor("bq", [1, 384], BF16, kind="ExternalInput")
    Wo = nc.dram_tensor("Wo", [128, 1024], BF16, kind="ExternalInput")
    ctab_d = nc.dram_tensor("ctab", [128, S], BF16, kind="ExternalInput")
    stab_d = nc.dram_tensor("stab", [128, S], BF16, kind="ExternalInput")
    ptab_d = nc.dram_tensor("ptab", [128, 128], BF16, kind="ExternalInput")
    out_d = nc.dram_tensor("out", [S2, D], BF16, kind="ExternalOutput")

    xT_re = xT.rearrange("(kb p) n -> p kb n", p=128)   # [128, 8, 4096]
    W_re = W.rearrange("(kb p) m -> p kb m", p=128)     # [128, 8, 384]

    with TileContext(nc) as tc:
        with tc.tile_pool(name="consts", bufs=1) as cp, \
             tc.tile_pool(name="xg", bufs=6) as xgp, \
             tc.tile_pool(name="pre", bufs=2) as prep, \
             tc.tile_pool(name="tmp", bufs=4) as tmpp, \
             tc.tile_pool(name="vt", bufs=2) as vtp, \
             tc.tile_pool(name="vv", bufs=2) as vvp, \
             tc.tile_pool(name="pa", bufs=3) as ptp, \
             tc.tile_pool(name="mrg", bufs=2) as mrgp, \
             tc.tile_pool(name="la", bufs=2) as lap, \
             tc.tile_pool(name="rc", bufs=2) as rcp, \
             tc.tile_pool(name="rlb", bufs=2) as rlbp, \
             tc.tile_pool(name="a2", bufs=2) as a2p, \
             tc.tile_pool(name="osb", bufs=2) as osbp, \
             tc.tile_pool(name="ps5", bufs=2, space="PSUM") as qkps, \
             tc.tile_pool(name="pssc", bufs=2, space="PSUM") as pssc, \
             tc.tile_pool(name="gam", bufs=1, space="PSUM") as gamp:

            W_r = cp.tile([128, 8 * 384], BF16, tag="W_r")
            Wo_r = cp.tile([128, 1024], BF16, tag="Wo_r")
            ctab = cp.tile([128, S], BF16, tag="ctab")
            stab = cp.tile([128, S], BF16, tag="stab")
            ptab = cp.tile([128, 128], BF16, tag="ptab")
            bq_r = cp.tile([128, 384], BF16, tag="bq_r")
            ones_r = cp.tile([128, G], BF16, tag="ones_r")
            qTb = [cp.tile([128, S], BF16, tag=f"qT{b}", name=f"qT{b}")
                   for b in range(B)]
            kTb = [cp.tile([128, S], BF16, tag=f"kT{b}", name=f"kT{b}")
                   for b in range(B)]
            V2b = [cp.tile([128, NTB * VG], BF16, tag=f"V2{b}", name=f"V2{b}")
                   for b in range(B)]

            nc.sync.dma_start(
                out=W_r[:].rearrange("p (kb m) -> p kb m", kb=8),
                in_=W_re[:])
            nc.scalar.dma_start(out=ctab[:], in_=ctab_d[:])
            nc.scalar.dma_start(out=stab[:], in_=stab_d[:])
            nc.scalar.dma_start(out=ptab[:], in_=ptab_d[:])
            nc.scalar.dma_start(out=Wo_r[:], in_=Wo[:])

            nc.gpsimd.memset(bq_r[:], 0.0)
            nc.scalar.dma_start(out=bq_r[0:1, :], in_=bq[:])
            nc.gpsimd.memset(ones_r[:], 0.0)
            nc.gpsimd.memset(ones_r[0:1, :], 1.0)
            for b in range(B):
                v2v = V2b[b][:].rearrange("p (g c) -> p g c", g=NTB)
                nc.gpsimd.memset(v2v[:, :, 64:65], 1.0)
                nc.gpsimd.memset(v2v[:, :, 65:66], 0.0)
                nc.gpsimd.memset(v2v[:, :, 66:67], 1.0)
                nc.gpsimd.memset(v2v[:, :, 67:130], 0.0)

            # ---------------- emitters ----------------------------------
            def emit_xg_load(b, g, eng=None):
                tok0 = b * S + g * G
                xg = xgp.tile([128, 8 * G], BF16, tag="xg", name=f"xg{b}_{g}")
                (eng or nc.sync).dma_start(
                    out=xg[:].rearrange("p (kb n) -> p kb n", kb=8),
                    in_=xT_re[:, :, tok0:tok0 + G])
                return xg

            def emit_qkv_mt(b, g, xg, mt, defer_v=None):
                scol = g * G
                ps = qkps.tile([128, G], F32, tag="ps5", name=f"qkv{b}{g}{mt}")
                for kb in range(8):
                    c0 = kb * 384 + mt * 128
                    nc.tensor.matmul(
                        ps[:], W_r[:, c0:c0 + 128], xg[:, kb * G:(kb + 1) * G],
                        start=(kb == 0), stop=False)
                nc.tensor.matmul(
                    ps[:], bq_r[:, mt * 128:(mt + 1) * 128], ones_r[:],
                    start=False, stop=True)
                if mt < 2:
                    pre = prep.tile([128, G], BF16, tag="pre",
                                    name=f"pre{b}{g}{mt}")
                    nc.vector.tensor_copy(pre[:], ps[:])
                    rot = qkps.tile([128, G], F32, tag="ps5",
                                    name=f"rot{b}{g}{mt}")
                    nc.tensor.matmul(rot[:], ptab[:], pre[:],
                                     start=True, stop=True)
                    t1 = tmpp.tile([128, G], BF16, tag="tmp",
                                   name=f"t1{b}{g}{mt}")
                    nc.vector.tensor_tensor(
                        out=t1[:], in0=rot[:], in1=stab[:, scol:scol + G],
                        op=MUL)
                    t2 = tmpp.tile([128, G], BF16, tag="tmp",
                                   name=f"t2{b}{g}{mt}")
                    nc.vector.tensor_tensor(
                        out=t2[:], in0=pre[:], in1=ctab[:, scol:scol + G],
                        op=MUL)
                    dst = (qTb if mt == 0 else kTb)[b]
                    nc.vector.tensor_tensor(
                        out=dst[:, scol:scol + G], in0=t1[:], in1=t2[:],
                        op=ADD)
                else:
                    vt = vtp.tile([128, G], BF16, tag="vt", name=f"vt{b}{g}")
                    nc.vector.tensor_copy(vt[:], ps[:])

                    def emit_vplace(b=b, g=g, vt=vt, eng=None):
                        vv = vvp.tile([128, G], BF16, tag="vv",
                                      name=f"vv{b}{g}")
                        (eng or nc.sync).dma_start(
                            out=vv[:].rearrange("p (g c) -> p g c", g=4),
                            in_=vt[:], transpose=True)
                        vvv = vv[:].rearrange("p (g c) -> p g c", g=4)
                        v2v = V2b[b][:].rearrange("p (g c) -> p g c", g=NTB)
                        g4 = g * 4
                        nc.vector.tensor_copy(
                            v2v[:, g4:g4 + 4, 0:64], vvv[:, :, 0:64])
                        nc.vector.tensor_copy(
                            v2v[:, g4:g4 + 4, 130:194], vvv[:, :, 64:128])
                    if defer_v is not None:
                        defer_v.append(emit_vplace)
                    else:
                        emit_vplace()

            heavy = []          # batch-1 qkv emitters (~2us PE each)
            light = []          # outproj units + stores (~0.5us each)
            HEAVY_SLOTS = frozenset((3, 7, 11))
            LIGHT_SLOTS = frozenset((2, 5, 9, 13, 15))

            def emit_sc(b, qcol, qw, uid, stage_at=(), box=None):
                if stage_at:
                    gen = _emit_sc_gen(b, qcol, qw, uid, stage_at, box)
                    return gen
                g = _emit_sc_gen(b, qcol, qw, uid, (), box)
                for _ in g:
                    pass
                return box[0] if box else None

            def _emit_sc_gen(b, qcol, qw, uid, stage_at, box):
                qT, kT, V2 = qTb[b], kTb[b], V2b[b]
                gam = gamp.tile([128, 1024], F32, tag="gam",
                                name=f"gam{uid}")

                def av(tb, pa):
                    gcol = tb * VG
                    st, sp = (tb == 0), (tb == NTB - 1)
                    nc.tensor.matmul(
                        gam[0:65, 0:qw], V2[:, gcol:gcol + 65],
                        pa[:, 0:qw], start=st, stop=sp)
                    nc.tensor.matmul(
                        gam[:, 512:512 + qw], V2[:, gcol + 66:gcol + 194],
                        pa[:, 512:512 + qw], start=st, stop=sp)

                prev = None
                for tb in range(NTB):
                    if tb in stage_at:
                        yield tb
                    tcol = tb * 128
                    sco = pssc.tile([128, 1024], F32, tag="sco",
                                    name=f"sco{uid}{tb}")
                    nc.tensor.matmul(
                        sco[:, 0:qw], kT[0:64, tcol:tcol + 128],
                        qT[0:64, qcol:qcol + qw], start=True, stop=True)
                    nc.tensor.matmul(
                        sco[:, 512:512 + qw], kT[64:128, tcol:tcol + 128],
                        qT[64:128, qcol:qcol + qw], start=True, stop=True)
                    pa = ptp.tile([128, 1024], BF16, tag="pa",
                                  name=f"pa{uid}{tb}")
                    scv = sco[:].rearrange("p (h q) -> p h q", h=2)
                    pav = pa[:].rearrange("p (h q) -> p h q", h=2)
                    nc.scalar.activation(
                        pav[:, :, 0:qw], scv[:, :, 0:qw], EXP, scale=0.125)
                    if prev is not None:
                        av(*prev)
                    if tb in HEAVY_SLOTS and heavy:
                        heavy.pop(0)()
                    elif (tb in LIGHT_SLOTS or not heavy) and light \
                            and tb >= (2 if heavy else 4):
                        light.pop(0)()
                    prev = (tb, pa)
                av(*prev)

                # merge: attnA rows 0:64 (l_A at row 64, cols 0:qw),
                # attnB rows 64:128 (l_B at row 0, cols 512:512+qw)
                s_t = mrgp.tile([128, 1024], F32, tag="s_t",
                                name=f"s_t{uid}")
                nc.vector.tensor_copy(s_t[:, 0:qw], gam[:, 0:qw])
                nc.vector.tensor_copy(
                    s_t[:, 512:512 + qw], gam[:, 512:512 + qw])
                la0 = lap.tile([1, 512], F32, tag="la", name=f"la{uid}")
                nc.sync.dma_start(out=la0[0:1, 0:qw], in_=s_t[64:65, 0:qw])
                rc = rcp.tile([1, 1024], F32, tag="rc", name=f"rc{uid}")
                nc.vector.reciprocal_approx_fast(
                    out=rc[0:1, 0:qw], in_=la0[0:1, 0:qw])
                nc.vector.reciprocal_approx_fast(
                    out=rc[0:1, 512:512 + qw], in_=s_t[0:1, 512:512 + qw])
                rlb = rlbp.tile([128, 1024], F32, tag="rlb",
                                name=f"rlb{uid}")
                rcv = rc[:].rearrange("p (h q) -> p h q", h=2)
                rlbv = rlb[:].rearrange("p (h q) -> p h q", h=2)
                nc.gpsimd.partition_broadcast(
                    out_ap=rlbv[:, :, 0:qw], in_ap=rcv[:, :, 0:qw])
                a2 = a2p.tile([128, 512], BF16, tag="a2", name=f"a2{uid}")
                nc.vector.tensor_tensor(
                    out=a2[0:64, 0:qw], in0=s_t[0:64, 0:qw],
                    in1=rlb[0:64, 0:qw], op=MUL)
                nc.vector.tensor_tensor(
                    out=a2[64:128, 0:qw], in0=s_t[64:128, 512:512 + qw],
                    in1=rlb[64:128, 512:512 + qw], op=MUL)
                if box is not None:
                    box.append(a2)
                yield NTB

            def make_outproj(b, qcol, qw, a2, uid):
                nnb = qw // 128
                osb = osbp.tile([128, 4 * 1024], BF16, tag="osb",
                                name=f"osb{uid}")
                osbv = osb[:].rearrange("p (g c) -> p g c", g=4)

                def emit_nb(nb):
                    for jc in range(2):
                        om = qkps.tile([128, 512], F32, tag="ps5",
                                       name=f"om{uid}{nb}{jc}")
                        nc.tensor.matmul(
                            om[:], a2[:, nb * 128:(nb + 1) * 128],
                            Wo_r[:, jc * 512:(jc + 1) * 512],
                            start=True, stop=True)
                        nc.vector.tensor_copy(
                            osbv[:, nb, jc * 512:(jc + 1) * 512], om[:])

                def emit_store():
                    r0 = b * S + qcol
                    nc.sync.dma_start(
                        out=out_d[r0:r0 + qw, :].rearrange(
                            "(g p) d -> p g d", p=128),
                        in_=osbv[:, 0:nnb, :])

                return [lambda nb=nb: emit_nb(nb) for nb in range(nnb)] \
                    + [emit_store]

            # ---------------- schedule ----------------------------------
            # batch 0 projections up front; xg0 rides the scalar HWDGE
            # queue so it transfers in parallel with W on the sync queue
            # batch 0 projections up front
            xg_next = emit_xg_load(0, 0)
            for g in range(GPB):
                xg = xg_next
                if g + 1 < GPB:
                    xg_next = emit_xg_load(0, g + 1)
                for mt in range(3):
                    emit_qkv_mt(0, g, xg, mt)

            # prefetch batch 1 token groups
            xg1 = [emit_xg_load(1, g) for g in range(GPB)]
            for g in range(GPB):
                for mt in range(3):
                    heavy.append(
                        lambda g=g, mt=mt: emit_qkv_mt(1, g, xg1[g], mt))

            # attention: batch 0 interleaves batch-1 projections and its
            # own output projections; the final query chunks shrink so
            # the closing merge+outproj tail is short
            chunks = [(0, sc * 512, 512) for sc in range(NSC)] \
                + [(1, sc * 512, 512) for sc in range(NSC - 1)] \
                + [(1, 1536, 256), (1, 1792, 256)]
            for i, (b, qcol, qw) in enumerate(chunks):
                box = []
                emit_sc(b, qcol, qw, uid=f"c{i}", box=box)
                light.extend(make_outproj(b, qcol, qw, box[0], uid=f"c{i}"))

            for fn in heavy + light:
                fn()
            heavy.clear()
            light.clear()

    nc.compile()
    return nc


_PROG = None


def _get_program():
    global _PROG
    if _PROG is None:
        _PROG = _build_program()
    return _PROG


def _rope_tables():
    inv_freq = (1.0 / (10000.0 ** (np.arange(0, DH, 2, dtype=np.float32) / DH)))
    invf2 = inv_freq[np.arange(128) % 32]
    ang = np.arange(S, dtype=np.float32)[None, :] * invf2[:, None]
    return np.cos(ang).astype(BF), np.sin(ang).astype(BF)


def _ptab():
    p = np.zeros((128, 128), dtype=np.float32)
    j = np.arange(128)
    p[j ^ 32, j] = np.where((j % 64) < 32, -1.0, 1.0)
    return p.astype(BF)


def make_in_maps(x, W_qkv, b_qkv, W_out, b_out):
    x = np.asarray(x, dtype=np.float32)
    W_qkv = np.asarray(W_qkv, dtype=np.float32)
    b_qkv = np.asarray(b_qkv, dtype=np.float32)
    W_out = np.asarray(W_out, dtype=np.float32)

    xT = np.ascontiguousarray(x.reshape(S2, D).T).astype(BF)
    ct, st = _rope_tables()
    pt = _ptab()

    in_maps = []
    for c in range(8):
        hA, hB = 2 * c, 2 * c + 1
        cols = np.r_[hA * DH:(hA + 1) * DH, hB * DH:(hB + 1) * DH]
        Wc = np.ascontiguousarray(
            np.concatenate([W_qkv[:, off + cols] for off in (0, D, 2 * D)],
                           axis=1)).astype(BF)
        bqc = np.concatenate(
            [b_qkv[off + cols] for off in (0, D, 2 * D)])[None, :].astype(BF)
        Woc = np.ascontiguousarray(W_out[c * 128:(c + 1) * 128, :]).astype(BF)
        in_maps.append({"xT": xT, "W": Wc, "bq": bqc, "Wo": Woc,
                       "ctab": ct, "stab": st, "ptab": pt})
    return in_maps


def assemble_output(results, b_out):
    acc = np.asarray(results[0]["out"]).astype(np.float32)
    for c in range(1, 8):
        acc += np.asarray(results[c]["out"]).astype(np.float32)
    out = acc + np.asarray(b_out, dtype=np.float32)
    return out.reshape(B, S, D).astype(np.float32)


def kernel(x, W_qkv, b_qkv, W_out, b_out):
    nc = _get_program()
    in_maps = make_in_maps(x, W_qkv, b_qkv, W_out, b_out)
    res = run_bass_kernel_spmd(nc, in_maps, core_ids=list(range(8)))
    return assemble_output(res.results, b_out)


if __name__ == "__main__":
    rng = np.random.default_rng(0)
    ins = {
        "x": rng.standard_normal((B, S, D), dtype=np.float32),
        "W_qkv": rng.standard_normal((D, 3 * D), dtype=np.float32) / 32.0,
        "b_qkv": np.zeros(3 * D, np.float32),
        "W_out": rng.standard_normal((D, D), dtype=np.float32) / 32.0,
        "b_out": np.zeros(D, np.float32),
    }
    o = kernel(**ins)
    print("kernel ran:", o.shape, o.dtype)


# revision 13
# speedup vs baseline: 1.2484x; 1.0014x over previous
"""Multi-head attention (B=2, S=2048, D=1024, H=16, RoPE) on 8 Trainium2 cores.

Sharding: tensor-parallel over heads. Core c owns heads (2c, 2c+1):
 - W_qkv column-sliced to that head pair (q|k|v blocks of 128 cols each),
 - W_out row-sliced to the pair's 128 input dims,
 - every core reads all tokens (x shipped pre-transposed as x^T, bf16),
 - each core emits a partial [4096, 1024] bf16 output; host sums the 8
   partials in f32 and adds b_out (Megatron-style allreduce on host).

Device program (per core, identical SPMD; all matmul operands bf16 so the
PE streams 1 row/cycle and LDWEIGHTS uses fast-weight-load):
  QKV runs in 512-token groups with the weight slice stationary across the
  whole group (9 accumulating matmuls incl. a ones-row bias matmul). RoPE =
  ptab-matmul rotate + two DVE multiply-adds against bf16 cos/sin tables.
  V^T is produced by the DMA XBAR transpose into a staging tile, then two
  strided DVE copies place it in the V2 layout
  [VA(64) | 1 | pad | 1 | 0(63) | VB(64)] (group width 194) so the two
  attn@V matmuls per key block also produce the softmax denominators:
  l_A lands on PSUM row 64 (cols 0:512), l_B on row 0 (cols 512:1024).
  Attention per (batch, 512-query chunk) pipelines 128-key blocks:
  score matmuls use 64-partition operands (no zero-padded K tiles), one
  batched exp on ACT with the 1/8 scale folded in, attn@V accumulated in
  PSUM. The merge avoids any DRAM bounce: one tiny SBUF->SBUF DMA hops
  l_A to partition 0, reciprocal_approx_fast + one partition_broadcast
  give 1/l, and two bf16 multiplies build the outproj stationary a2.
  Batch-1 QKV and the output projections are interleaved into the
  attention key-block slots through a task queue so the PE never idles.
"""

import sys

if "/opt/trn_rl_repo" not in sys.path:
    sys.path.insert(0, "/opt/trn_rl_repo")

import numpy as np
import ml_dtypes

import concourse.bacc as bacc
import concourse.mybir as mybir
from concourse.tile import TileContext
from concourse.bass_utils import run_bass_kernel_spmd

F32 = mybir.dt.float32
BF16 = mybir.dt.bfloat16
BF = ml_dtypes.bfloat16
ADD = mybir.AluOpType.add
MUL = mybir.AluOpType.mult
EXP = mybir.ActivationFunctionType.Exp

B, S, D, H, DH = 2, 2048, 1024, 16, 64
S2 = B * S              # 4096 tokens total
G = 512                 # token group for the projection phase
GPB = S // G            # 4 groups per batch
NSC = 4                 # 512-query chunks per batch
NTB = S // 128          # 16 key blocks per batch
VG = 194                # V2 group: VA(64)|1|pad|1|zeros(63)|VB(64)


def _build_program(has_bias=True):
    nc = bacc.Bacc("TRN2", target_bir_lowering=False, debug=False, num_devices=8)

    xT = nc.dram_tensor("xT", [D, S2], BF16, kind="ExternalInput")
    W = nc.dram_tensor("W", [D, 384], BF16, kind="ExternalInput")
    bq = (nc.dram_tensor("bq", [1, 384], BF16, kind="ExternalInput")
          if has_bias else None)
    Wo = nc.dram_tensor("Wo", [128, 1024], BF16, kind="ExternalInput")
    ctab_d = nc.dram_tensor("ctab", [128, S], BF16, kind="ExternalInput")
    stab_d = nc.dram_tensor("stab", [128, S], BF16, kind="ExternalInput")
    ptab_d = nc.dram_tensor("ptab", [128, 128], BF16, kind="ExternalInput")
    out_d = nc.dram_tensor("out", [S2, D], BF16, kind="ExternalOutput")

    xT_re = xT.rearrange("(kb p) n -> p kb n", p=128)   # [128, 8, 4096]
    W_re = W.rearrange("(kb p) m -> p kb m", p=128)     # [128, 8, 384]

    with TileContext(nc) as tc:
        with tc.tile_pool(name="consts", bufs=1) as cp, \
             tc.tile_pool(name="xg", bufs=6) as xgp, \
             tc.tile_pool(name="pre", bufs=2) as prep, \
             tc.tile_pool(name="tmp", bufs=4) as tmpp, \
             tc.tile_pool(name="vt", bufs=2) as vtp, \
             tc.tile_pool(name="vv", bufs=2) as vvp, \
             tc.tile_pool(name="pa", bufs=3) as ptp, \
             tc.tile_pool(name="mrg", bufs=2) as mrgp, \
             tc.tile_pool(name="la", bufs=2) as lap, \
             tc.tile_pool(name="rc", bufs=2) as rcp, \
             tc.tile_pool(name="rlb", bufs=2) as rlbp, \
             tc.tile_pool(name="a2", bufs=2) as a2p, \
             tc.tile_pool(name="osb", bufs=2) as osbp, \
             tc.tile_pool(name="ps5", bufs=2, space="PSUM") as qkps, \
             tc.tile_pool(name="pssc", bufs=2, space="PSUM") as pssc, \
             tc.tile_pool(name="gam", bufs=1, space="PSUM") as gamp:

            W_r = cp.tile([128, 8 * 384], BF16, tag="W_r")
            Wo_r = cp.tile([128, 1024], BF16, tag="Wo_r")
            ctab = cp.tile([128, S], BF16, tag="ctab")
            stab = cp.tile([128, S], BF16, tag="stab")
            ptab = cp.tile([128, 128], BF16, tag="ptab")
            if has_bias:
                bq_r = cp.tile([128, 384], BF16, tag="bq_r")
                ones_r = cp.tile([128, G], BF16, tag="ones_r")
            qTb = [cp.tile([128, S], BF16, tag=f"qT{b}", name=f"qT{b}")
                   for b in range(B)]
            kTb = [cp.tile([128, S], BF16, tag=f"kT{b}", name=f"kT{b}")
                   for b in range(B)]
            V2b = [cp.tile([128, NTB * VG], BF16, tag=f"V2{b}", name=f"V2{b}")
                   for b in range(B)]

            nc.sync.dma_start(
                out=W_r[:].rearrange("p (kb m) -> p kb m", kb=8),
                in_=W_re[:])
            nc.scalar.dma_start(out=ctab[:], in_=ctab_d[:])
            nc.scalar.dma_start(out=stab[:], in_=stab_d[:])
            nc.scalar.dma_start(out=ptab[:], in_=ptab_d[:])
            nc.scalar.dma_start(out=Wo_r[:], in_=Wo[:])

            if has_bias:
                nc.gpsimd.memset(bq_r[:], 0.0)
                nc.scalar.dma_start(out=bq_r[0:1, :], in_=bq[:])
                nc.gpsimd.memset(ones_r[:], 0.0)
                nc.gpsimd.memset(ones_r[0:1, :], 1.0)
            for b in range(B):
                v2v = V2b[b][:].rearrange("p (g c) -> p g c", g=NTB)
                nc.gpsimd.memset(v2v[:, :, 64:65], 1.0)
                nc.gpsimd.memset(v2v[:, :, 65:66], 0.0)
                nc.gpsimd.memset(v2v[:, :, 66:67], 1.0)
                nc.gpsimd.memset(v2v[:, :, 67:130], 0.0)

            # ---------------- emitters ----------------------------------
            def emit_xg_load(b, g, eng=None):
                tok0 = b * S + g * G
                xg = xgp.tile([128, 8 * G], BF16, tag="xg", name=f"xg{b}_{g}")
                (eng or nc.sync).dma_start(
                    out=xg[:].rearrange("p (kb n) -> p kb n", kb=8),
                    in_=xT_re[:, :, tok0:tok0 + G])
                return xg

            def emit_qkv_mt(b, g, xg, mt, defer_v=None):
                scol = g * G
                ps = qkps.tile([128, G], F32, tag="ps5", name=f"qkv{b}{g}{mt}")
                for kb in range(8):
                    c0 = kb * 384 + mt * 128
                    nc.tensor.matmul(
                        ps[:], W_r[:, c0:c0 + 128], xg[:, kb * G:(kb + 1) * G],
                        start=(kb == 0), stop=(kb == 7 and not has_bias))
                if has_bias:
                    nc.tensor.matmul(
                        ps[:], bq_r[:, mt * 128:(mt + 1) * 128], ones_r[:],
                        start=False, stop=True)
                if mt < 2:
                    pre = prep.tile([128, G], BF16, tag="pre",
                                    name=f"pre{b}{g}{mt}")
                    nc.vector.tensor_copy(pre[:], ps[:])
                    rot = qkps.tile([128, G], F32, tag="ps5",
                                    name=f"rot{b}{g}{mt}")
                    nc.tensor.matmul(rot[:], ptab[:], pre[:],
                                     start=True, stop=True)
                    t1 = tmpp.tile([128, G], BF16, tag="tmp",
                                   name=f"t1{b}{g}{mt}")
                    nc.vector.tensor_tensor(
                        out=t1[:], in0=rot[:], in1=stab[:, scol:scol + G],
                        op=MUL)
                    t2 = tmpp.tile([128, G], BF16, tag="tmp",
                                   name=f"t2{b}{g}{mt}")
                    nc.vector.tensor_tensor(
                        out=t2[:], in0=pre[:], in1=ctab[:, scol:scol + G],
                        op=MUL)
                    dst = (qTb if mt == 0 else kTb)[b]
                    nc.vector.tensor_tensor(
                        out=dst[:, scol:scol + G], in0=t1[:], in1=t2[:],
                        op=ADD)
                else:
                    vt = vtp.tile([128, G], BF16, tag="vt", name=f"vt{b}{g}")
                    nc.vector.tensor_copy(vt[:], ps[:])

                    def emit_vplace(b=b, g=g, vt=vt, eng=None):
                        vv = vvp.tile([128, G], BF16, tag="vv",
                                      name=f"vv{b}{g}")
                        (eng or nc.sync).dma_start(
                            out=vv[:].rearrange("p (g c) -> p g c", g=4),
                            in_=vt[:], transpose=True)
                        vvv = vv[:].rearrange("p (g c) -> p g c", g=4)
                        v2v = V2b[b][:].rearrange("p (g c) -> p g c", g=NTB)
                        g4 = g * 4
                        nc.vector.tensor_copy(
                            v2v[:, g4:g4 + 4, 0:64], vvv[:, :, 0:64])
                        nc.vector.tensor_copy(
                            v2v[:, g4:g4 + 4, 130:194], vvv[:, :, 64:128])
                    if defer_v is not None:
                        defer_v.append(emit_vplace)
                    else:
                        emit_vplace()

            heavy = []          # batch-1 qkv emitters (~2us PE each)
            light = []          # outproj units + stores (~0.5us each)
            HEAVY_SLOTS = frozenset((3, 7, 11))
            LIGHT_SLOTS = frozenset((2, 5, 9, 13, 15))

            def emit_sc(b, qcol, qw, uid, stage_at=(), box=None):
                if stage_at:
                    gen = _emit_sc_gen(b, qcol, qw, uid, stage_at, box)
                    return gen
                g = _emit_sc_gen(b, qcol, qw, uid, (), box)
                for _ in g:
                    pass
                return box[0] if box else None

            def _emit_sc_gen(b, qcol, qw, uid, stage_at, box):
                qT, kT, V2 = qTb[b], kTb[b], V2b[b]
                gam = gamp.tile([128, 1024], F32, tag="gam",
                                name=f"gam{uid}")

                def av(tb, pa):
                    gcol = tb * VG
                    st, sp = (tb == 0), (tb == NTB - 1)
                    nc.tensor.matmul(
                        gam[0:65, 0:qw], V2[:, gcol:gcol + 65],
                        pa[:, 0:qw], start=st, stop=sp)
                    nc.tensor.matmul(
                        gam[:, 512:512 + qw], V2[:, gcol + 66:gcol + 194],
                        pa[:, 512:512 + qw], start=st, stop=sp)

                prev = None
                for tb in range(NTB):
                    if tb in stage_at:
                        yield tb
                    tcol = tb * 128
                    sco = pssc.tile([128, 1024], F32, tag="sco",
                                    name=f"sco{uid}{tb}")
                    nc.tensor.matmul(
                        sco[:, 0:qw], kT[0:64, tcol:tcol + 128],
                        qT[0:64, qcol:qcol + qw], start=True, stop=True)
                    nc.tensor.matmul(
                        sco[:, 512:512 + qw], kT[64:128, tcol:tcol + 128],
                        qT[64:128, qcol:qcol + qw], start=True, stop=True)
                    pa = ptp.tile([128, 1024], BF16, tag="pa",
                                  name=f"pa{uid}{tb}")
                    scv = sco[:].rearrange("p (h q) -> p h q", h=2)
                    pav = pa[:].rearrange("p (h q) -> p h q", h=2)
                    nc.scalar.activation(
                        pav[:, :, 0:qw], scv[:, :, 0:qw], EXP, scale=0.125)
                    if prev is not None:
                        av(*prev)
                    if tb in HEAVY_SLOTS and heavy:
                        heavy.pop(0)()
                    elif (tb in LIGHT_SLOTS or not heavy) and light \
                            and tb >= (2 if heavy else 4):
                        light.pop(0)()
                    prev = (tb, pa)
                av(*prev)

                # merge: attnA rows 0:64 (l_A at row 64, cols 0:qw),
                # attnB rows 64:128 (l_B at row 0, cols 512:512+qw)
                s_t = mrgp.tile([128, 1024], F32, tag="s_t",
                                name=f"s_t{uid}")
                nc.vector.tensor_copy(s_t[:, 0:qw], gam[:, 0:qw])
                nc.vector.tensor_copy(
                    s_t[:, 512:512 + qw], gam[:, 512:512 + qw])
                la0 = lap.tile([1, 512], F32, tag="la", name=f"la{uid}")
                nc.sync.dma_start(out=la0[0:1, 0:qw], in_=s_t[64:65, 0:qw])
                rc = rcp.tile([1, 1024], F32, tag="rc", name=f"rc{uid}")
                nc.vector.reciprocal_approx_fast(
                    out=rc[0:1, 0:qw], in_=la0[0:1, 0:qw])
                nc.vector.reciprocal_approx_fast(
                    out=rc[0:1, 512:512 + qw], in_=s_t[0:1, 512:512 + qw])
                rlb = rlbp.tile([128, 1024], F32, tag="rlb",
                                name=f"rlb{uid}")
                rcv = rc[:].rearrange("p (h q) -> p h q", h=2)
                rlbv = rlb[:].rearrange("p (h q) -> p h q", h=2)
                nc.gpsimd.partition_broadcast(
                    out_ap=rlbv[:, :, 0:qw], in_ap=rcv[:, :, 0:qw])
                a2 = a2p.tile([128, 512], BF16, tag="a2", name=f"a2{uid}")
                nc.vector.tensor_tensor(
                    out=a2[0:64, 0:qw], in0=s_t[0:64, 0:qw],
                    in1=rlb[0:64, 0:qw], op=MUL)
                nc.vector.tensor_tensor(
                    out=a2[64:128, 0:qw], in0=s_t[64:128, 512:512 + qw],
                    in1=rlb[64:128, 512:512 + qw], op=MUL)
                if box is not None:
                    box.append(a2)
                yield NTB

            def make_outproj(b, qcol, qw, a2, uid):
                nnb = qw // 128
                osb = osbp.tile([128, 4 * 1024], BF16, tag="osb",
                                name=f"osb{uid}")
                osbv = osb[:].rearrange("p (g c) -> p g c", g=4)

                def emit_nb(nb):
                    for jc in range(2):
                        om = qkps.tile([128, 512], F32, tag="ps5",
                                       name=f"om{uid}{nb}{jc}")
                        nc.tensor.matmul(
                            om[:], a2[:, nb * 128:(nb + 1) * 128],
                            Wo_r[:, jc * 512:(jc + 1) * 512],
                            start=True, stop=True)
                        nc.vector.tensor_copy(
                            osbv[:, nb, jc * 512:(jc + 1) * 512], om[:])

                def emit_store():
                    r0 = b * S + qcol
                    nc.sync.dma_start(
                        out=out_d[r0:r0 + qw, :].rearrange(
                            "(g p) d -> p g d", p=128),
                        in_=osbv[:, 0:nnb, :])

                return [lambda nb=nb: emit_nb(nb) for nb in range(nnb)] \
                    + [emit_store]

            # ---------------- schedule ----------------------------------
            # batch 0 projections up front; xg0 rides the scalar HWDGE
            # queue so it transfers in parallel with W on the sync queue
            # batch 0 projections up front
            xg_next = emit_xg_load(0, 0)
            for g in range(GPB):
                xg = xg_next
                if g + 1 < GPB:
                    xg_next = emit_xg_load(0, g + 1)
                for mt in range(3):
                    emit_qkv_mt(0, g, xg, mt)

            # prefetch batch 1 token groups
            xg1 = [emit_xg_load(1, g) for g in range(GPB)]
            for g in range(GPB):
                for mt in range(3):
                    heavy.append(
                        lambda g=g, mt=mt: emit_qkv_mt(1, g, xg1[g], mt))

            # attention: batch 0 interleaves batch-1 projections and its
            # own output projections; the final query chunks shrink so
            # the closing merge+outproj tail is short
            chunks = [(0, sc * 512, 512) for sc in range(NSC)] \
                + [(1, sc * 512, 512) for sc in range(NSC - 1)] \
                + [(1, 1536, 256), (1, 1792, 256)]
            for i, (b, qcol, qw) in enumerate(chunks):
                box = []
                emit_sc(b, qcol, qw, uid=f"c{i}", box=box)
                light.extend(make_outproj(b, qcol, qw, box[0], uid=f"c{i}"))

            for fn in heavy + light:
                fn()
            heavy.clear()
            light.clear()

    nc.compile()
    return nc


_PROGS = {}


def _get_program(has_bias=False):
    if has_bias not in _PROGS:
        _PROGS[has_bias] = _build_program(has_bias)
    return _PROGS[has_bias]


def _rope_tables():
    inv_freq = (1.0 / (10000.0 ** (np.arange(0, DH, 2, dtype=np.float32) / DH)))
    invf2 = inv_freq[np.arange(128) % 32]
    ang = np.arange(S, dtype=np.float32)[None, :] * invf2[:, None]
    return np.cos(ang).astype(BF), np.sin(ang).astype(BF)


def _ptab():
    p = np.zeros((128, 128), dtype=np.float32)
    j = np.arange(128)
    p[j ^ 32, j] = np.where((j % 64) < 32, -1.0, 1.0)
    return p.astype(BF)


def make_in_maps(x, W_qkv, b_qkv, W_out, b_out, has_bias=None):
    x = np.asarray(x, dtype=np.float32)
    W_qkv = np.asarray(W_qkv, dtype=np.float32)
    b_qkv = np.asarray(b_qkv, dtype=np.float32)
    W_out = np.asarray(W_out, dtype=np.float32)

    if has_bias is None:
        has_bias = bool(np.any(b_qkv))
    xT = np.ascontiguousarray(x.reshape(S2, D).T).astype(BF)
    ct, st = _rope_tables()
    pt = _ptab()

    in_maps = []
    for c in range(8):
        hA, hB = 2 * c, 2 * c + 1
        cols = np.r_[hA * DH:(hA + 1) * DH, hB * DH:(hB + 1) * DH]
        Wc = np.ascontiguousarray(
            np.concatenate([W_qkv[:, off + cols] for off in (0, D, 2 * D)],
                           axis=1)).astype(BF)
        Woc = np.ascontiguousarray(W_out[c * 128:(c + 1) * 128, :]).astype(BF)
        m = {"xT": xT, "W": Wc, "Wo": Woc,
             "ctab": ct, "stab": st, "ptab": pt}
        if has_bias:
            m["bq"] = np.concatenate(
                [b_qkv[off + cols]
                 for off in (0, D, 2 * D)])[None, :].astype(BF)
        in_maps.append(m)
    return in_maps


def assemble_output(results, b_out):
    acc = np.asarray(results[0]["out"]).astype(np.float32)
    for c in range(1, 8):
        acc += np.asarray(results[c]["out"]).astype(np.float32)
    out = acc + np.asarray(b_out, dtype=np.float32)
    return out.reshape(B, S, D).astype(np.float32)


def kernel(x, W_qkv, b_qkv, W_out, b_out):
    has_bias = bool(np.any(np.asarray(b_qkv)))
    nc = _get_program(has_bias)
    in_maps = make_in_maps(x, W_qkv, b_qkv, W_out, b_out, has_bias=has_bias)
    res = run_bass_kernel_spmd(nc, in_maps, core_ids=list(range(8)))
    return assemble_output(res.results, b_out)


if __name__ == "__main__":
    rng = np.random.default_rng(0)
    ins = {
        "x": rng.standard_normal((B, S, D), dtype=np.float32),
        "W_qkv": rng.standard_normal((D, 3 * D), dtype=np.float32) / 32.0,
        "b_qkv": np.zeros(3 * D, np.float32),
        "W_out": rng.standard_normal((D, D), dtype=np.float32) / 32.0,
        "b_out": np.zeros(D, np.float32),
    }
    o = kernel(**ins)
    print("kernel ran:", o.shape, o.dtype)
